# revision 4
# baseline (speedup 1.0000x reference)
"""TT-adapter linear kernel for TRN2, data-parallel over batch on 8 NeuronCores.

Math: out = x @ W.T + b + ALPHA * TT(x).  TT is linear in x, so the module
collapses to a single matmul with a merged weight folded on host:

    Wc = W + ALPHA * T          (T = TT-matrix reconstruction, 1024x1024)
    out = x @ Wc.T + b

The 34 GFLOP batched matmul runs on device in bf16 (f32 PSUM accumulation),
one batch element per NeuronCore, no collectives.  Raw bacc (manual
semaphores).  PE floor is 256 MMs x 216 ns = 55.3 us.

Measured DMA behavior that shapes the schedule: one HWDGE queue sustains
~300 GB/s early, ~400 GB/s warm, and a DMA's completion semaphore reaches
+16 only ~0.3 us (small DMA) to ~1.4 us (512 KB w/ backlog) after its first
engine finishes -- so every PE gate costs data-time + sem-spread.  Hence:
small DMAs for anything on the critical path, big DMAs for bulk.

Host layouts (per core, P=128 partitions, contraction dim on partitions):
    wt  bf16 [8, 128, 1024]     wt[d, p, o*128+j] = Wc[o*128+j, 128d+p]
    xs  bf16 [4, 128, 8, 512]   xs[sc, p, d, j]   = x[b, 512sc+j, 128d+p]
    bi  f32  [128, 8]           bi[p, oo]         = b[128oo + p]
    out bf16 [8, 128, 2048]     out[oo, p, s]     = result[b, s, 128oo+p]

Schedule per core (group idx = 8*sc + o; bank = o; all MMs N=512):
  SP:  inputs in arrival-critical order: per-d weight halves (128 KB each,
       small sem-spread) interleaved with the sc=0 x slices, then the bulk
       sc=1..3 x (1 MB each) and bias; then out-DMAs idx=0..30 gated on
       evictions; final wait on the 8 slot sems.
  PE:  10 HAM-warm-up matmuls; phase 1 = sc=0 strip (o=0..7) d-outer
       staircase across all 8 PSUM banks -- needs only 384 KB per d step;
       phase 2 = sc=1..3 strips, d-inner per group, gated on that sc's x
       bulk DMA + bank eviction.
  ACT: dummy 8-col activate first (hoists the lazy 1.3 us ACT_TABLE_LOAD
       into the preamble), then 32 evictions (PSUM -> SBUF bf16 + bias),
       last group's out-DMA ships from ACT directly.
"""

import numpy as np
import ml_dtypes
from contextlib import ExitStack

import concourse.bass as bass  # noqa: F401
import concourse.mybir as mybir
from concourse import bacc
from concourse.bass_utils import run_bass_kernel_spmd

ALPHA = 16.0
B, S, D = 8, 2048, 1024
P = 128
DO = D // P          # 8 contraction tiles
OO = D // P          # 8 output tiles
SCH = 512
NS = S // SCH        # 4 s-chunks
NG = OO * NS         # 32 groups
NBANK = 8
NSLOT = 8

_NC = None


def _build_nc():
    nc = bacc.Bacc("TRN2", target_bir_lowering=False, debug=False)
    wt = nc.declare_dram_parameter("wt", [DO, P, D], mybir.dt.bfloat16, isOutput=False)
    xs = nc.declare_dram_parameter("xs", [NS, P, DO, SCH], mybir.dt.bfloat16, isOutput=False)
    bi = nc.declare_dram_parameter("bi", [P, OO], mybir.dt.float32, isOutput=False)
    out = nc.declare_dram_parameter("out", [OO, P, S], mybir.dt.bfloat16, isOutput=True)

    with ExitStack() as ctx:
        block = ctx.enter_context(nc.Block())
        # One sem per gating granule; HWDGE completion increments +16 per DMA
        # but the 16 per-engine incs straggle, so granules stay small where
        # latency matters.
        s_wa = [ctx.enter_context(nc.semaphore(f"s_wa{d}")) for d in range(DO)]
        s_wb = [ctx.enter_context(nc.semaphore(f"s_wb{d}")) for d in range(DO)]
        s_x0a = ctx.enter_context(nc.semaphore("s_x0a"))    # xs[0][:, 0:2]
        s_x0b = ctx.enter_context(nc.semaphore("s_x0b"))    # xs[0][:, 2:8]
        s_xs = [ctx.enter_context(nc.semaphore(f"s_xs{sc}")) for sc in range(1, NS)]
        s_bias = ctx.enter_context(nc.semaphore("s_bias"))
        s_mm = ctx.enter_context(nc.semaphore("s_mm"))
        s_ev = ctx.enter_context(nc.semaphore("s_ev"))
        s_slot = [ctx.enter_context(nc.semaphore(f"s_slot{k}")) for k in range(NSLOT)]

        w_sb = ctx.enter_context(nc.sbuf_tensor("w_sb", [P, DO, D], mybir.dt.bfloat16))
        x_sb = ctx.enter_context(nc.sbuf_tensor("x_sb", [P, NS, DO, SCH], mybir.dt.bfloat16))
        bias_sb = ctx.enter_context(nc.sbuf_tensor("bias_sb", [P, OO], mybir.dt.float32))
        ot_sb = ctx.enter_context(nc.sbuf_tensor("ot_sb", [P, NSLOT, SCH], mybir.dt.bfloat16))
        ps = [ctx.enter_context(nc.psum_tensor(f"ps{b}", [P, SCH], mybir.dt.float32))
              for b in range(NBANK)]

        def wsl(o, d):
            return w_sb[:, d, o * P:(o + 1) * P]

        def xsl(sc, d):
            return x_sb[:, sc, d, :]

        @block.sync
        def _(sync: bass.BassEngine):
            H = D // 2
            # critical path: d=0 weights (two 128KB halves) + x(sc0, d=0,1)
            sync.dma_start(out=w_sb[:, 0, 0:H], in_=wt[0][:, 0:H]).then_inc(s_wa[0], 16)
            sync.dma_start(out=x_sb[:, 0, 0:2, :], in_=xs[0][:, 0:2, :]).then_inc(s_x0a, 16)
            sync.dma_start(out=w_sb[:, 0, H:D], in_=wt[0][:, H:D]).then_inc(s_wb[0], 16)
            sync.dma_start(out=w_sb[:, 1, 0:H], in_=wt[1][:, 0:H]).then_inc(s_wa[1], 16)
            sync.dma_start(out=w_sb[:, 1, H:D], in_=wt[1][:, H:D]).then_inc(s_wb[1], 16)
            sync.dma_start(out=x_sb[:, 0, 2:DO, :], in_=xs[0][:, 2:DO, :]).then_inc(s_x0b, 16)
            for d in range(2, DO):
                sync.dma_start(out=w_sb[:, d, 0:H], in_=wt[d][:, 0:H]).then_inc(s_wa[d], 16)
                sync.dma_start(out=w_sb[:, d, H:D], in_=wt[d][:, H:D]).then_inc(s_wb[d], 16)
            for sc in range(1, NS):
                sync.dma_start(out=x_sb[:, sc, :, :], in_=xs[sc]).then_inc(s_xs[sc - 1], 16)
            sync.dma_start(out=bias_sb[:, :], in_=bi[:, :]).then_inc(s_bias, 16)
            for g in range(NG - 1):
                o, sc = g % OO, g // OO
                sync.wait_ge(s_ev, g + 1)
                sync.dma_start(
                    out=out[o, :, sc * SCH:(sc + 1) * SCH],
                    in_=ot_sb[:, g % NSLOT, :],
                ).then_inc(s_slot[g % NSLOT], 16)
            for k in range(NSLOT):
                sync.wait_ge(s_slot[k], 16 * (NG // NSLOT))

        @block.tensor
        def _(tensor: bass.BassEngine):
            # HAM warm-up on whatever is in SBUF during the otherwise-idle
            # preamble/input-latency window; results discarded (group o=0
            # restarts bank 0 with start=True).
            for _ in range(10):
                tensor.matmul(
                    ps[0][:, 0:256],
                    w_sb[:, 0, 0:P],
                    x_sb[:, 0, 0, 0:256],
                    start=True,
                    stop=True,
                )
            # phase 1: sc=0 strip, d-outer staircase over banks 0..7 (=o)
            for d in range(DO):
                if d == 0:
                    tensor.wait_ge(s_wa[0], 16)
                    tensor.wait_ge(s_x0a, 16)
                elif d == 2:
                    tensor.wait_ge(s_x0b, 16)
                    tensor.wait_ge(s_wa[d], 16)
                else:
                    tensor.wait_ge(s_wa[d], 16)
                for o in range(OO):
                    if o == OO // 2:
                        tensor.wait_ge(s_wb[d], 16)
                    mmi = tensor.matmul(
                        ps[o][:, :],
                        wsl(o, d),
                        xsl(0, d),
                        start=(d == 0),
                        stop=(d == DO - 1),
                    )
                    if d == DO - 1:
                        # d=7 octet runs in group order 0..7 -> s_mm incs
                        # arrive in the order the evictions expect
                        mmi.then_inc(s_mm, 1)
            # phase 2: sc=1..3 strips, d-inner per group
            for g in range(NBANK, NG):
                o, sc = g % OO, g // OO
                if o == 0:
                    tensor.wait_ge(s_xs[sc - 1], 16)
                tensor.wait_ge(s_ev, g - NBANK + 1)
                for d in range(DO):
                    mmi = tensor.matmul(
                        ps[o][:, :],
                        wsl(o, d),
                        xsl(sc, d),
                        start=(d == 0),
                        stop=(d == DO - 1),
                    )
                    if d == DO - 1:
                        mmi.then_inc(s_mm, 1)

        @block.scalar
        def _(scalar: bass.BassEngine):
            # dummy 8-col activate: pulls the lazy ACT_TABLE_LOAD into the
            # preamble window (it otherwise delays the first real eviction
            # by ~1.3us).  Reads garbage; slot 0 is fully overwritten by
            # eviction 0 before any out-DMA reads it.
            scalar.add(ot_sb[:, 0, 0:8], bias_sb[:, 0:8], 0.0)
            for g in range(NG):
                o, sc = g % OO, g // OO
                if g == 0:
                    scalar.wait_ge(s_bias, 16)
                scalar.wait_ge(s_mm, g + 1)
                if g >= NSLOT:
                    scalar.wait_ge(s_slot[g % NSLOT], 16 * (g // NSLOT))
                scalar.add(
                    ot_sb[:, g % NSLOT, :], ps[o][:, :], bias_sb[:, o:o + 1]
                ).then_inc(s_ev, 1)
                if g == NG - 1:
                    # last output ships from ACT (also HWDGE, its own queue):
                    # skips the SP semaphore hop on the critical tail
                    scalar.dma_start(
                        out=out[o, :, sc * SCH:(sc + 1) * SCH],
                        in_=ot_sb[:, g % NSLOT, :],
                    ).then_inc(s_slot[g % NSLOT], 16)

    nc.compile()
    return nc


def _get_nc():
    global _NC
    if _NC is None:
        _NC = _build_nc()
    return _NC


def _merged_weight_T(W, b, core0, core1, core2, core3, core4, core5):
    f8 = np.float64
    A = core0[0].astype(f8)
    Bm = np.einsum('ap,pbq->abq', A, core1.astype(f8))
    C = np.einsum('abq,qcr->abcr', Bm, core2.astype(f8))
    Phi = C.transpose(2, 1, 0, 3).reshape(D, 8)
    Dn = np.einsum('paq,qbr->pabr', core3.astype(f8), core4.astype(f8))
    E = np.einsum('pabq,qc->pabc', Dn, core5[:, :, 0].astype(f8))
    Psi = E.reshape(8, D)
    WcT = W.T.astype(f8) + ALPHA * (Phi @ Psi)
    return WcT.astype(np.float32)


def _prep_in_maps(x, W, b, core0, core1, core2, core3, core4, core5):
    WcT = _merged_weight_T(W, b, core0, core1, core2, core3, core4, core5)
    wt = WcT.reshape(DO, P, D).astype(ml_dtypes.bfloat16)
    bi = np.ascontiguousarray(b.reshape(OO, P).T).astype(np.float32)
    in_maps = []
    for bb in range(B):
        xt = x[bb].T.reshape(DO, P, NS, SCH)
        xsc = np.ascontiguousarray(xt.transpose(2, 1, 0, 3)).astype(ml_dtypes.bfloat16)
        in_maps.append({"wt": wt, "xs": xsc, "bi": bi})
    return in_maps


def _gather(results):
    outs = []
    for bb in range(B):
        o = np.asarray(results[bb]["out"]).astype(np.float32)
        outs.append(o.transpose(2, 0, 1).reshape(S, D))
    return np.ascontiguousarray(np.stack(outs))


def run(inputs, **spmd_kwargs):
    inputs = {k: np.asarray(v) for k, v in inputs.items()}
    in_maps = _prep_in_maps(**inputs)
    nc = _get_nc()
    res = run_bass_kernel_spmd(nc, in_maps, core_ids=list(range(B)), **spmd_kwargs)
    return _gather(res.results), res


def kernel(x, W, b, core0, core1, core2, core3, core4, core5):
    out, _ = run(dict(x=x, W=W, b=b, core0=core0, core1=core1, core2=core2,
                      core3=core3, core4=core4, core5=core5))
    return out
